# revision 29
# baseline (speedup 1.0000x reference)
"""AttackNet kernel for 8 Trainium2 NeuronCores (v3: bf16 DVE pipeline).

Reference computation:
    out  = conv1x1(x, W) + b                        # 60 channels
    pert = out.reshape(n, 20, 3, h, w)[arange, target]
    pert = ((pert - min) / (max - min) - 0.5) * 2   # per (sample, channel) spatial
    return pert * (MAX_PERTURBATION / 128)

Only the 3 gathered channels per sample matter, and the bias cancels in
the min/max normalization. For target-class weights W[j, 0..2] (j = out
channel) the device computes, per sample:
    q       = x0 * r1 + x1          r1 = W[j,0]/W[j,1]   (Vector STT, bf16 2x)
    lin_pre = q  * r2 + x2          r2 = W[j,1]/W[j,2]   (Vector LINSTAT)
            = (W[j,0] x0 + W[j,1] x1 + W[j,2] x2) / W[j,2]
    out     = lin_pre * s_eff + t                        (Scalar ACT / Vector TS)
with s_eff = 2*sigma/R, t = -(MX+MN)*sigma/R  (sigma = sign(W[j,2]),
R = MX-MN) from the spatial stats MX/MN of lin_pre.  LINSTAT emits the
per-row max via a scan-max into a sentinel pad column and the per-row
min via its accumulator; a PE transpose + grouped reduces + a tiny
[R|P] matmul collapse rows to per-sample stats.

Everything on-chip is bf16 (mixed-dtype DVE ops hit microcode slow
paths; bf16 same-dtype runs 2x); the stats path is f32 after one tiny
cast-copy.  HBM IO is bf16 both ways (tolerance 2e-2).
Sharding: pure data parallel, 4 samples per core across 8 cores.
Per-core layout: partition p = sample*32 + spatial_block, 1568 cols
+ 1 pad col per partition.
"""

import sys
import time

sys.path.insert(0, "/opt/trn_rl_repo")
sys.path.insert(0, "/root/problem")

import numpy as np
from ml_dtypes import bfloat16

import concourse.bass as bass  # noqa: F401
import concourse.tile as tile
from concourse import bacc, mybir
from concourse.bass_utils import run_bass_kernel_spmd


def _install_ntff_hook_shim():
    """Provide antenv.axon_hooks (absent in this image) so trace=True works."""
    import types

    if "antenv.axon_hooks" in sys.modules:
        return
    import contextlib
    import ctypes

    so_path = "/opt/axon/libaxon_pjrt.so"
    try:
        lib = ctypes.CDLL(so_path)
        lib.axon_start_nrt_profile.argtypes = [
            ctypes.POINTER(ctypes.c_int64),
            ctypes.c_size_t,
        ]
        lib.axon_start_nrt_profile.restype = ctypes.c_int64
        lib.axon_stop_nrt_profile.argtypes = [ctypes.c_char_p]
        lib.axon_stop_nrt_profile.restype = ctypes.c_int64
    except OSError:
        lib = None

    @contextlib.contextmanager
    def _hook(output_dir, device_ids):
        import jax

        jax.devices()
        if device_ids:
            ids = (ctypes.c_int64 * len(device_ids))(*device_ids)
            rc = lib.axon_start_nrt_profile(ids, len(device_ids))
        else:
            rc = lib.axon_start_nrt_profile(None, 0)
        if rc != 0:
            raise RuntimeError(f"axon_start_nrt_profile rc={rc}")
        try:
            yield
        finally:
            n = lib.axon_stop_nrt_profile(str(output_dir).encode())
            print(f"ntff profile: {n} file(s) written to {output_dir}",
                  file=sys.stderr)

    mod = types.ModuleType("antenv.axon_hooks")
    mod.get_axon_ntff_profile_hook = lambda: (_hook if lib is not None else None)
    mod.set_axon_ntff_profile_hook = lambda h: None
    import antenv

    antenv.axon_hooks = mod
    sys.modules["antenv.axon_hooks"] = mod


_install_ntff_hook_shim()

# --- custom DVE op: lin = in0*s0 + in1, scan-max -> pad col, min -> accum ---
from concourse import dve_ops
from concourse.dve_spec import (
    AluOp, C0, C1, C2, Spec, Src0, Src1, lower, scan, select,
)
from concourse.dve_uop import DveOpSpec


def _linstat_ref(in0, in1, c0, c1, c2):
    v = (in0 * c0 + in1).astype(np.float32)
    r = np.maximum.accumulate(v, axis=-1)
    o = np.where(in1 <= c1, r, v)
    acc = np.minimum(
        np.float32(c2), o.reshape(o.shape[0], -1).min(-1, keepdims=True)
    )
    return o, acc


def _register(name, spec):
    for op in dve_ops.OPS:
        if op.name == name:
            return op
    opcode = dve_ops._CUSTOM_DVE_ROW_BASE + len(dve_ops.OPS)
    assert opcode < 0x20
    shas = {}
    for ver in ("v3", "v4"):
        uops = lower(spec, ver=ver)
        shas[ver] = DveOpSpec(
            name=name, opcode=opcode, uops=uops, rd1_en=True
        ).sha(ver)
    op = dve_ops.DveOp(name, spec, subdim=False, uops_sha=shas)
    dve_ops.OPS.append(op)
    dve_ops.CUSTOM_DVE_SPECS[name] = spec
    dve_ops._SUB_OPCODE_FOR_NAME[name] = opcode
    return op


_v = Src0 * C0 + Src1
LINSTAT = _register(
    "LINSTAT_ATK",
    Spec(
        body=select(Src1 <= C1, scan(AluOp.MAX, _v), _v),
        accum=AluOp.MIN,
        accum_init=C2,
        reference=_linstat_ref,
    ),
)

P = 128                 # SBUF partitions
H = W_ = 224
F = H * W_              # 50176 spatial elements per plane
G32 = 32                # partitions per sample group
NS = 4                  # samples per core
FD = F // G32           # 1568 free elements per partition
NCORES = 8
N = NCORES * NS         # 32 samples total
OUT_CHANNELS = 3
PAD_SENTINEL = -3.0e38
# aux f32 [128, AUXW]: cols 0-2 r1_j | 3-5 r2_j | 6-8 c2sig_j (rows 0:4)
#   | 9-10 M = [[1,-1],[1,1]] cols for [R|P] matmul (rows 0:2)
#   | 11..11+128 gmat (rows 0:4)
AUXW = 11 + P

_CACHE = {}


def _build():
    f32 = mybir.dt.float32
    bf = mybir.dt.bfloat16
    mult = mybir.AluOpType.mult
    add = mybir.AluOpType.add
    mx = mybir.AluOpType.max
    ident = mybir.ActivationFunctionType.Identity

    nc = bacc.Bacc(
        "TRN2", target_bir_lowering=False, debug=False, num_devices=1
    )
    # host-padded, partition-major: xs[c, p, :] with p = sample*32 + block
    xs01 = nc.dram_tensor("xs01", [P, 2, FD + 2], bf, kind="ExternalInput")
    xs2 = nc.dram_tensor("xs2", [P, FD + 2], bf, kind="ExternalInput")
    aux = nc.dram_tensor("aux", [P, AUXW], f32, kind="ExternalInput")
    identf = nc.dram_tensor("identf", [P, P], f32, kind="ExternalInput")
    out = nc.dram_tensor("out", [3, P, FD], bf, kind="ExternalOutput")

    with tile.TileContext(nc) as tc:
        with (
            tc.tile_pool(name="wp", bufs=1) as wp,
            tc.tile_pool(name="xp", bufs=1) as xp,
            tc.tile_pool(name="qp", bufs=2) as qp,
            tc.tile_pool(name="lp", bufs=3) as lp,
            tc.tile_pool(name="st", bufs=3) as st,
            tc.tile_pool(name="pp", bufs=2, space="PSUM") as pp,
            tc.tile_pool(name="op", bufs=3) as outp,
        ):
            x01 = xp.tile([P, 2, FD + 2], bf, tag="x01")
            x2 = xp.tile([P, FD + 2], bf, tag="x2")
            auxt = wp.tile([P, AUXW], f32, tag="aux")
            identt = wp.tile([P, P], f32, tag="identf")
            nc.scalar.dma_start(auxt[:], aux[:])
            nc.sync.dma_start(x01[:], xs01[:])
            nc.scalar.dma_start(x2[:], xs2[:])
            nc.scalar.dma_start(identt[:], identf[:])
            x0 = x01[:, 0]
            x1 = x01[:, 1]

            r1 = lambda j: auxt[:, j : j + 1]            # noqa: E731
            r2 = lambda j: auxt[:, 3 + j : 4 + j]        # noqa: E731
            c2s = lambda j: auxt[0:NS, 6 + j : 7 + j]    # noqa: E731
            rpm = auxt[0:2, 9:11]
            gmat = auxt[0:NS, 11 : 11 + P]

            qs, lins, stsbs = {}, {}, {}

            def emit_q(j):
                qs[j] = qp.tile([P, FD + 2], bf, name=f"q{j}", tag=f"q{j}")
                nc.vector.scalar_tensor_tensor(
                    qs[j][:], x0, r1(j), x1, op0=mult, op1=add
                )

            def emit_lin(j):
                lins[j] = lp.tile(
                    [P, FD + 3], bf, name=f"lin{j}", tag=f"lin{j}"
                )
                nc.vector._custom_dve(
                    LINSTAT,
                    out=lins[j][:, 0 : FD + 2],
                    in0=qs[j][:],
                    in1=x2[:],
                    s0=r2(j),
                    s1=-1.0e38,
                    imm2=3.4e38,
                    accum_out=lins[j][:, FD + 2 : FD + 3],
                )

            def emit_stats(j):
                # cast stat cols [max|min] to f32, transpose to [2, 128],
                # grouped reduces -> [2, 4] = per-sample MX / MN
                with tc.high_priority():
                    stf = st.tile([P, 2], f32, tag="stf")
                    nc.vector.tensor_copy(
                        stf[:, 0:1], lins[j][:, FD + 1 : FD + 2]
                    )
                    nc.vector.tensor_scalar_mul(
                        stf[:, 1:2], lins[j][:, FD + 2 : FD + 3], -1.0
                    )
                    ps1 = pp.tile([2, P], f32, tag="ps1")
                    nc.tensor.transpose(ps1[:], stf[:], identt[:])
                    st4 = st.tile([2, NS], f32, tag="st4")
                    nc.vector.tensor_reduce(
                        st4[:], ps1[:].rearrange("r (n g) -> r n g", g=G32),
                        axis=mybir.AxisListType.X, op=mx,
                    )
                    # rows of st4: [MX, -MN]
                    # [R|P] = st4^T . [[1,1],[1,-1]] (R = MX-MN, P_ = MX+MN)
                    ps2 = pp.tile([NS, 2], f32, tag="ps2")
                    nc.tensor.matmul(
                        ps2[:], st4[:], rpm, start=True, stop=True
                    )
                    # s_eff = c2sig/R ; t = P_*s_eff*(-0.5)
                    inv = st.tile([NS, 1], f32, tag="inv")
                    nc.vector.reciprocal(inv[:], ps2[:, 0:1])
                    st2 = st.tile([NS, 2], f32, tag="st2")
                    nc.vector.tensor_scalar_mul(st2[:, 0:1], inv[:], c2s(j))
                    nc.vector.tensor_scalar(
                        st2[:, 1:2], ps2[:, 1:2], st2[:, 0:1], -0.5,
                        op0=mult, op1=mult,
                    )
                    # broadcast per-sample [s_eff|t] to all 128 partitions;
                    # norm ops read scale/bias straight from PSUM
                    ps3 = pp.tile([P, 2], f32, tag="ps3")
                    nc.tensor.matmul(
                        ps3[:], gmat, st2[:], start=True, stop=True
                    )
                    stsbs[j] = ps3

            def emit_norm_scalar(j):
                ot = outp.tile([P, FD], bf, name=f"ot{j}", tag=f"ot{j}")
                nc.scalar.activation(
                    ot[:], lins[j][:, 0:FD], ident,
                    bias=stsbs[j][:, 1:2], scale=stsbs[j][:, 0:1],
                )
                nc.sync.dma_start(out[j], ot[:])

            def emit_norm_vector(j):
                ot = outp.tile([P, FD], bf, name=f"ot{j}", tag=f"ot{j}")
                nc.vector.tensor_scalar(
                    ot[:], lins[j][:, 0:FD],
                    stsbs[j][:, 0:1], stsbs[j][:, 1:2],
                    op0=mult, op1=add,
                )
                # spread output DMAs over both DGE rings so their per-row
                # descriptor generation overlaps
                ring = nc.scalar if j == 1 else nc.sync
                ring.dma_start(out[j], ot[:])

            emit_q(0)
            emit_lin(0)
            emit_q(1)
            emit_stats(0)
            emit_lin(1)
            emit_norm_vector(0)
            emit_q(2)
            emit_stats(1)
            emit_lin(2)
            emit_norm_vector(1)
            emit_stats(2)
            emit_norm_vector(2)

    nc.compile()
    return nc


def get_nc():
    if "nc" not in _CACHE:
        _CACHE["nc"] = _build()
    return _CACHE["nc"]


def make_in_maps(x, target, W, b):
    x = np.ascontiguousarray(np.asarray(x), dtype=np.float32)
    tgt = np.asarray(target).astype(np.int64)
    Wm = np.asarray(W, dtype=np.float32).reshape(20 * OUT_CHANNELS, 3)
    Wsel = Wm.reshape(20, OUT_CHANNELS, 3)[tgt]  # (N, 3 out, 3 in)

    w0 = Wsel[:, :, 0]  # (N, 3j)
    w1 = Wsel[:, :, 1].copy()
    w2 = Wsel[:, :, 2].copy()
    eps = 1e-30
    w1[np.abs(w1) < eps] = eps
    w2[np.abs(w2) < eps] = eps
    r1 = (w0 / w1).astype(np.float32)          # (N, 3)
    r2v = (w1 / w2).astype(np.float32)         # (N, 3)
    c2s = (2.0 * np.sign(w2)).astype(np.float32)

    # x -> [N, 3, 32, 1568] bf16 + pad col (sentinel on channel 2)
    xr = x.reshape(N, 3, G32, FD)
    xpad = np.zeros((N, 3, G32, FD + 2), dtype=bfloat16)
    xpad[..., :FD] = xr.astype(bfloat16)
    xpad[:, 2, :, FD:] = bfloat16(PAD_SENTINEL)

    eye = np.eye(P, dtype=np.float32)
    in_maps = []
    for core in range(NCORES):
        lo = core * NS
        xsc = np.ascontiguousarray(
            xpad[lo : lo + NS].transpose(1, 0, 2, 3).reshape(3, P, FD + 2)
        )
        xs01m = np.ascontiguousarray(
            xsc[0:2].transpose(1, 0, 2)
        )  # [P, 2, FD+2]
        xs2m = np.ascontiguousarray(xsc[2])
        auxm = np.zeros((P, AUXW), dtype=np.float32)
        auxm[:, 0:3] = np.repeat(r1[lo : lo + NS], G32, axis=0)
        auxm[:, 3:6] = np.repeat(r2v[lo : lo + NS], G32, axis=0)
        auxm[0:NS, 6:9] = c2s[lo : lo + NS]
        auxm[0:2, 9:11] = np.array([[1.0, 1.0], [1.0, -1.0]],
                                   dtype=np.float32)
        auxm[0:NS, 11 : 11 + P] = np.repeat(
            np.eye(NS, dtype=np.float32), G32, axis=1
        )
        in_maps.append(
            {"xs01": xs01m, "xs2": xs2m, "aux": auxm, "identf": eye}
        )
    return in_maps


def run(x, target, W, b, trace=False, retries=2):
    nc = get_nc()
    in_maps = make_in_maps(x, target, W, b)
    last_err = None
    for attempt in range(retries + 1):
        try:
            res = run_bass_kernel_spmd(
                nc, in_maps, list(range(NCORES)), trace=trace
            )
            outs = []
            for r in res.results:
                o = np.asarray(r["out"]).astype(np.float32)
                o = o.reshape(3, NS, G32, FD).transpose(1, 0, 2, 3)
                outs.append(o.reshape(NS, OUT_CHANNELS, H, W_))
            return np.concatenate(outs, axis=0), res
        except Exception as e:  # device may need recovery; retry
            last_err = e
            if attempt < retries:
                time.sleep(20)
    raise last_err


def kernel(x, target, W, b):
    out, _ = run(x, target, W, b)
    return out
